# revision 1
# baseline (speedup 1.0000x reference)
"""Trainium2 Bass kernel for nn_CombNetHE (dual-MLP + HE-friendly soft blend).

Contract: kernel(**inputs) takes FULL unsharded fp32 inputs (numpy or jax
arrays) and returns the FULL [16384, 10] fp32 output. Internally shards the
batch across 8 NeuronCores (2048 rows each), runs a fused Tile kernel per
core, and gathers on the host.

Per-core layout:
  - x is pre-transposed/cast on host to xT [D_IN, M] bf16 so layer 1 can run
    with W1 as the stationary operand and xT as the 512-wide moving operand,
    producing h TRANSPOSED (hT[dh, m]) in PSUM. That makes hT directly usable
    as the stationary operand of layer 2 (logits[m, 10]) with no on-chip
    transposes anywhere.
  - relu(h + b1) + bf16 cast is a single ScalarE activation per [128, 512]
    tile, PSUM -> SBUF.
  - softmax / comp_max_tau / blend run in fp32 on DVE+ACT, batched over the
    whole 512-row block ([128 partitions x 4 chunks]).
"""

import os
import sys

for _p in ("/opt/trn_rl_repo", "/root/.axon_site/_ro/trn_rl_repo"):
    if os.path.isdir(_p) and _p not in sys.path:
        sys.path.insert(0, _p)

from contextlib import ExitStack

import ml_dtypes
import numpy as np

import concourse.bass as bass
import concourse.bacc as bacc
import concourse.mybir as mybir
import concourse.tile as tile
from concourse.bass_utils import run_bass_kernel_spmd

B, D_IN, D_H, C = 16384, 1024, 4096, 10
TAU, T1, T2 = 0.5, 3, 3
N_CORES = 8
M = B // N_CORES  # rows per core
M_BLK = 512  # rows processed per outer block
N_MBLK = M // M_BLK  # 4
MM = M_BLK // 128  # 4 partition-chunks per block
KC = D_IN // 128  # 8 contraction chunks (layer 1)
DC = D_H // 128  # 32 hidden chunks

F32 = mybir.dt.float32
BF16 = mybir.dt.bfloat16
Alu = mybir.AluOpType
Act = mybir.ActivationFunctionType

LAST_RESULTS = None
_BUILD_CACHE = {}


def _build_module(reps=1):
    nc = bacc.Bacc(
        "TRN2", target_bir_lowering=False, debug=False, num_devices=N_CORES
    )

    xT_d = nc.dram_tensor("xT", [D_IN, M], BF16, kind="ExternalInput")
    w1_d = {}
    w2_d = {}
    b1_d = {}
    b2_d = {}
    for n in ("o", "f"):
        w1_d[n] = nc.dram_tensor(f"w1{n}", [D_IN, D_H], BF16, kind="ExternalInput")
        w2_d[n] = nc.dram_tensor(f"w2{n}", [D_H, C], BF16, kind="ExternalInput")
        b1_d[n] = nc.dram_tensor(f"b1{n}", [D_H], F32, kind="ExternalInput")
        b2_d[n] = nc.dram_tensor(f"b2{n}", [C], F32, kind="ExternalInput")
    out_d = nc.dram_tensor("out", [M, C], F32, kind="ExternalOutput")

    with tile.TileContext(nc) as tc, ExitStack() as ctx:
        consts = ctx.enter_context(tc.tile_pool(name="consts", bufs=1))
        hpool = ctx.enter_context(tc.tile_pool(name="hpool", bufs=8))
        epool = ctx.enter_context(tc.tile_pool(name="epool", bufs=3))
        opool = ctx.enter_context(tc.tile_pool(name="opool", bufs=3))
        psum_h = ctx.enter_context(tc.tile_pool(name="psum_h", bufs=4, space="PSUM"))
        psum_o = ctx.enter_context(tc.tile_pool(name="psum_o", bufs=1, space="PSUM"))

        # ---- resident tensors -------------------------------------------
        # DMAs are emitted in consumption order so PE can start ~5us in:
        # xT(blk0), then W1o by dh-group, small tensors, W1f, xT(blk1..3).
        GS = 512  # dh elements per weight group tile
        NG = D_H // GS  # 8 groups
        DC_G = GS // 128  # dh chunks per group

        # xT: per (blk, kc) tiles [128, M_BLK] bf16, k = kc*128 + p
        xT_sb = [[None] * KC for _ in range(N_MBLK)]

        def load_x_blk(blk):
            for kc in range(KC):
                t = consts.tile(
                    [128, M_BLK], BF16, name=f"xT{blk}_{kc}", tag=f"xT{blk}_{kc}"
                )
                nc.sync.dma_start(
                    t[:],
                    xT_d.ap()[
                        kc * 128 : (kc + 1) * 128, blk * M_BLK : (blk + 1) * M_BLK
                    ],
                )
                xT_sb[blk][kc] = t

        # PE pre-warm: dummy matmuls on memset tiles run during the initial
        # weight DMA, so the clock-gate/p-state ramp burns idle time, not
        # real work. Uses a rotating ph slot; results are never read.
        warm_w = consts.tile([128, 128], BF16, name="warm_w", tag="warm_w")
        warm_x = consts.tile([128, M_BLK], BF16, name="warm_x", tag="warm_x")
        nc.vector.memset(warm_w[:], 0.0)
        nc.vector.memset(warm_x[:], 0.0)
        for _ in range(16):
            ph = psum_h.tile([128, M_BLK], F32, name="ph", tag="ph")
            nc.tensor.matmul(ph[:], lhsT=warm_w[:], rhs=warm_x[:])

        w1_sb = {n: [] for n in ("o", "f")}

        def load_w1_group(n, g):
            t = consts.tile(
                [128, KC, GS], BF16, name=f"w1{n}g{g}", tag=f"w1{n}g{g}"
            )
            nc.sync.dma_start(
                t[:],
                w1_d[n].ap()[:, g * GS : (g + 1) * GS].rearrange(
                    "(kc p) d -> p kc d", p=128
                ),
            )
            w1_sb[n].append(t)

        load_w1_group("o", 0)
        load_x_blk(0)

        w2_sb = {}
        b1_sb = {}
        b2_sb = {}
        for n in ("o", "f"):
            t = consts.tile([128, DC, C], BF16, name=f"w2{n}", tag=f"w2{n}")
            nc.sync.dma_start(
                t[:], w2_d[n].ap().rearrange("(dc p) c -> p dc c", p=128)
            )
            w2_sb[n] = t
            t = consts.tile([128, DC], F32, name=f"b1{n}", tag=f"b1{n}")
            nc.sync.dma_start(t[:], b1_d[n].ap().rearrange("(dc p) -> p dc", p=128))
            b1_sb[n] = t
            t = consts.tile([128, C], F32, name=f"b2{n}", tag=f"b2{n}")
            nc.sync.dma_start(
                t[:],
                bass.AP(tensor=b2_d[n], offset=0, ap=[[0, 128], [1, C]]),
            )
            b2_sb[n] = t
        for g in range(1, NG):
            load_w1_group("o", g)
        for g in range(NG):
            load_w1_group("f", g)

        for blk in range(1, N_MBLK):
            load_x_blk(blk)

        # ---- main loop ---------------------------------------------------
        for blk in range(N_MBLK * reps):
            blk = blk % N_MBLK
            m0 = blk * M_BLK
            probs = {}
            for n in ("o", "f"):
                # layer 1 + layer 2 fused over hidden chunks
                po = [
                    psum_o.tile([128, C], F32, name=f"po{mm}", tag=f"po{mm}")
                    for mm in range(MM)
                ]
                for dc in range(DC):
                    g, dl = dc // DC_G, dc % DC_G
                    ph = psum_h.tile([128, M_BLK], F32, name="ph", tag="ph")
                    for kc in range(KC):
                        nc.tensor.matmul(
                            ph[:],
                            lhsT=w1_sb[n][g][:, kc, dl * 128 : (dl + 1) * 128],
                            rhs=xT_sb[blk][kc][:],
                            start=(kc == 0),
                            stop=(kc == KC - 1),
                        )
                    hT = hpool.tile([128, M_BLK], BF16, name="hT", tag="hT")
                    nc.scalar.activation(
                        hT[:], ph[:], Act.Relu, bias=b1_sb[n][:, dc : dc + 1]
                    )
                    for mm in range(MM):
                        nc.tensor.matmul(
                            po[mm][:],
                            lhsT=hT[:, mm * 128 : (mm + 1) * 128],
                            rhs=w2_sb[n][:, dc, :],
                            start=(dc == 0),
                            stop=(dc == DC - 1),
                        )

                # softmax over C, batched [128, MM, C]
                z = epool.tile([128, MM, C], F32, name=f"z{n}", tag=f"z{n}")
                negmax = epool.tile([128, MM], F32, name=f"ngm{n}", tag=f"ngm{n}")
                exps = epool.tile([128, MM, C], F32, name=f"ex{n}", tag=f"ex{n}")
                sums = epool.tile([128, MM], F32, name=f"sm{n}", tag=f"sm{n}")
                rinv = epool.tile([128, MM], F32, name=f"ri{n}", tag=f"ri{n}")
                wrk = epool.tile([128, MM], F32, name=f"wk{n}", tag=f"wk{n}")
                for mm in range(MM):
                    nc.vector.tensor_tensor(
                        z[:, mm, :], po[mm][:], b2_sb[n][:], Alu.add
                    )
                for mm in range(MM):
                    nc.vector.tensor_reduce(
                        negmax[:, mm : mm + 1],
                        z[:, mm, :],
                        axis=mybir.AxisListType.X,
                        op=Alu.max,
                        negate=True,
                    )
                for mm in range(MM):
                    nc.scalar.activation(
                        exps[:, mm, :],
                        z[:, mm, :],
                        Act.Exp,
                        bias=negmax[:, mm : mm + 1],
                        accum_out=sums[:, mm : mm + 1],
                    )
                nc.vector.reciprocal(rinv[:], sums[:])
                # one Newton step: r <- r * (2 - s*r)
                nc.vector.tensor_tensor(wrk[:], sums[:], rinv[:], Alu.mult)
                nc.vector.tensor_scalar(wrk[:], wrk[:], -1.0, 2.0, Alu.mult, Alu.add)
                nc.vector.tensor_tensor(rinv[:], rinv[:], wrk[:], Alu.mult)
                pr = epool.tile([128, MM, C], F32, name=f"pr{n}", tag=f"pr{n}")
                nc.vector.tensor_tensor(
                    pr[:],
                    exps[:],
                    rinv[:, :, None].to_broadcast([128, MM, C]),
                    Alu.mult,
                )
                probs[n] = pr

            # ---- comp_max_tau on probs["o"] ------------------------------
            res = epool.tile([128, MM, C + 1], F32, name="res", tag="res")
            s4 = epool.tile([128, MM], F32, name="s4", tag="s4")
            u4 = epool.tile([128, MM], F32, name="u4", tag="u4")
            b4 = epool.tile([128, MM], F32, name="b4", tag="b4")
            a4 = epool.tile([128, MM], F32, name="a4", tag="a4")
            nc.vector.tensor_scalar(
                res[:, :, 0:C], probs["o"][:], 0.0, None, Alu.add
            )
            nc.vector.memset(res[:, :, C : C + 1], TAU)
            for i in range(T1):
                m_i = 2.0 + TAU * TAU if i == 0 else 2.0
                k_i = 2.0 / m_i
                nc.vector.tensor_tensor(res[:], res[:], res[:], Alu.mult)
                nc.vector.tensor_reduce(
                    s4[:], res[:], axis=mybir.AxisListType.X, op=Alu.add
                )
                nc.vector.tensor_scalar(u4[:], s4[:], k_i, None, Alu.mult)
                nc.vector.tensor_scalar(b4[:], u4[:], -1.0, 1.0, Alu.mult, Alu.add)
                nc.vector.tensor_scalar(
                    a4[:], u4[:], -k_i, 2.0 * k_i, Alu.mult, Alu.add
                )
                for _ in range(T2):
                    nc.vector.tensor_tensor(b4[:], b4[:], b4[:], Alu.mult)
                    nc.vector.scalar_tensor_tensor(
                        a4[:], b4[:], 1.0, a4[:], Alu.add, Alu.mult
                    )
                nc.vector.tensor_tensor(
                    res[:],
                    res[:],
                    a4[:, :, None].to_broadcast([128, MM, C + 1]),
                    Alu.mult,
                )

            # ---- blend: out = x1 + cond * (x2 - x1) ----------------------
            dd = epool.tile([128, MM, C], F32, name="dd", tag="dd")
            outt = opool.tile([128, MM, C], F32, name="outt", tag="outt")
            nc.vector.tensor_tensor(dd[:], probs["f"][:], probs["o"][:], Alu.subtract)
            nc.vector.tensor_tensor(
                dd[:],
                dd[:],
                res[:, :, C : C + 1].to_broadcast([128, MM, C]),
                Alu.mult,
            )
            nc.vector.tensor_tensor(outt[:], dd[:], probs["o"][:], Alu.add)
            nc.sync.dma_start(
                out_d.ap()[m0 : m0 + M_BLK, :].rearrange("(mm p) c -> p mm c", p=128),
                outt[:],
            )

    nc.compile()
    return nc


def _get_module():
    if "nc" not in _BUILD_CACHE:
        _BUILD_CACHE["nc"] = _build_module()
    return _BUILD_CACHE["nc"]


def kernel(x, W1o, b1o, W2o, b2o, W1f, b1f, W2f, b2f):
    x = np.asarray(x, dtype=np.float32)
    bf = ml_dtypes.bfloat16
    w1 = {
        "o": np.ascontiguousarray(np.asarray(W1o, np.float32).astype(bf)),
        "f": np.ascontiguousarray(np.asarray(W1f, np.float32).astype(bf)),
    }
    w2 = {
        "o": np.ascontiguousarray(np.asarray(W2o, np.float32).astype(bf)),
        "f": np.ascontiguousarray(np.asarray(W2f, np.float32).astype(bf)),
    }
    b1 = {
        "o": np.ascontiguousarray(np.asarray(b1o, np.float32)),
        "f": np.ascontiguousarray(np.asarray(b1f, np.float32)),
    }
    b2 = {
        "o": np.ascontiguousarray(np.asarray(b2o, np.float32)),
        "f": np.ascontiguousarray(np.asarray(b2f, np.float32)),
    }
    xb = x.astype(bf)

    nc = _get_module()

    in_maps = []
    for i in range(N_CORES):
        shard = np.ascontiguousarray(xb[i * M : (i + 1) * M, :].T)
        m = {"xT": shard}
        for n in ("o", "f"):
            m[f"w1{n}"] = w1[n]
            m[f"w2{n}"] = w2[n]
            m[f"b1{n}"] = b1[n]
            m[f"b2{n}"] = b2[n]
        in_maps.append(m)

    trace = bool(os.environ.get("KERNEL_TRACE"))
    results = run_bass_kernel_spmd(
        nc, in_maps, list(range(N_CORES)), trace=trace
    )
    global LAST_RESULTS
    LAST_RESULTS = results

    out = np.concatenate(
        [np.asarray(results.results[i]["out"], np.float32) for i in range(N_CORES)],
        axis=0,
    )
    return out



# revision 7
# speedup vs baseline: 1.5809x; 1.5809x over previous
"""Trainium2 Bass kernel for nn_CombNetHE (dual-MLP + HE-friendly soft blend).

Contract: kernel(**inputs) takes FULL unsharded fp32 inputs (numpy or jax
arrays) and returns the FULL [16384, 10] fp32 output. Internally shards the
batch across 8 NeuronCores (2048 rows each), runs a fused Tile kernel per
core, and gathers on the host.

Layer 1 runs on the PE in fp8-e4m3 with DoubleRow perf mode (2 stationary
planes per instruction, 0.5 cycles per output element -> 4x bf16 FLOP rate).
Accuracy is managed asymmetrically, informed by an offline error study:
  - net_orig (only feeds the soft cond indicator + the mostly-suppressed
    (1-cond) blend arm) runs pure fp8: h = x8 @ W8.
  - net_fake (dominates the output since cond~1 for most rows) gets two
    fp8 correction products per k-chunk: the weight-quantization residual
    (x8 @ e4m3(Ws - W8)) and the x-quantization residual (e4m3(x - x8) @ W8),
    recovering ~bf16-level h accuracy at fp8 speed.
W1 is pre-scaled by 32 on the host (std 1/32 -> ~1) to sit in e4m3's normal
range; the 1/32 descale folds into the ReLU activation's scale operand.

Per-core layout:
  - x is pre-transposed/quantized on host to x8T/xr8T [D_IN, M] e4m3 so layer
    1 runs with W1 stationary, producing h transposed (hT[dh, m]) in PSUM;
    hT (bf16) is then directly the stationary operand of layer 2.
  - relu(h/32 + b1) + bf16 cast is a single ScalarE activation per [128, 512]
    tile, PSUM -> SBUF.
  - softmax / comp_max_tau / blend run in fp32 on DVE+ACT, batched over the
    whole 512-row block ([128 partitions x 4 chunks]).
"""

import os
import sys

for _p in ("/opt/trn_rl_repo", "/root/.axon_site/_ro/trn_rl_repo"):
    if os.path.isdir(_p) and _p not in sys.path:
        sys.path.insert(0, _p)

from contextlib import ExitStack

import ml_dtypes
import numpy as np

import concourse.bass as bass
import concourse.bacc as bacc
import concourse.mybir as mybir
import concourse.tile as tile
from concourse.bass_utils import run_bass_kernel_spmd

B, D_IN, D_H, C = 16384, 1024, 4096, 10
TAU, T1, T2 = 0.5, 3, 3
N_CORES = 8
M = B // N_CORES  # rows per core
M_BLK = 512  # rows processed per outer block
N_MBLK = M // M_BLK  # 4
MM = M_BLK // 128  # 4 partition-chunks per block
KC = D_IN // 128  # 8 contraction chunks (layer 1)
DC = D_H // 128  # 32 hidden chunks
W_SCALE = 32.0  # host pre-scale on W1 before e4m3 quantization

# fp8 correction products per net: (cw, cx) = # of k-chunks getting the
# W-residual / x-residual correction matmuls (0..KC each).
CFG = {"o": (0, 0), "f": (KC, KC)}

F32 = mybir.dt.float32
BF16 = mybir.dt.bfloat16
FP8 = mybir.dt.float8e4
Alu = mybir.AluOpType
Act = mybir.ActivationFunctionType
DR = mybir.MatmulPerfMode.DoubleRow

LAST_RESULTS = None
_BUILD_CACHE = {}


def _build_module(reps=1):
    nc = bacc.Bacc(
        "TRN2", target_bir_lowering=False, debug=False, num_devices=N_CORES
    )

    x8_d = nc.dram_tensor("x8T", [D_IN, M], FP8, kind="ExternalInput")
    need_xr = any(cfg[1] > 0 for cfg in CFG.values())
    xr8_d = (
        nc.dram_tensor("xr8T", [D_IN, M], FP8, kind="ExternalInput")
        if need_xr
        else None
    )
    w1_d = {}
    wr1_d = {}
    w2_d = {}
    b1_d = {}
    b2_d = {}
    for n in ("o", "f"):
        w1_d[n] = nc.dram_tensor(f"w1{n}", [D_IN, D_H], FP8, kind="ExternalInput")
        if CFG[n][0] > 0:
            wr1_d[n] = nc.dram_tensor(
                f"wr1{n}", [D_IN, D_H], FP8, kind="ExternalInput"
            )
        w2_d[n] = nc.dram_tensor(f"w2{n}", [D_H, C], BF16, kind="ExternalInput")
        b1_d[n] = nc.dram_tensor(f"b1{n}", [D_H], F32, kind="ExternalInput")
        b2_d[n] = nc.dram_tensor(f"b2{n}", [C], F32, kind="ExternalInput")
    out_d = nc.dram_tensor("out", [M, C], F32, kind="ExternalOutput")

    with tile.TileContext(nc) as tc, ExitStack() as ctx:
        consts = ctx.enter_context(tc.tile_pool(name="consts", bufs=1))
        hpool = ctx.enter_context(tc.tile_pool(name="hpool", bufs=8))
        epool = ctx.enter_context(tc.tile_pool(name="epool", bufs=3))
        opool = ctx.enter_context(tc.tile_pool(name="opool", bufs=3))
        psum_h = ctx.enter_context(tc.tile_pool(name="psum_h", bufs=4, space="PSUM"))
        psum_o = ctx.enter_context(tc.tile_pool(name="psum_o", bufs=1, space="PSUM"))

        # ---- resident tensors -------------------------------------------
        # DMAs are emitted in consumption order so PE can start early.
        GS = 512  # dh elements per weight group tile
        NG = D_H // GS  # 8 groups
        DC_G = GS // 128  # 4 dh chunks per group

        # x8/xr8: per-blk tiles [128, KC, M_BLK] fp8; [p, kc, m] = x[kc*128+p, m]
        x8_sb = [None] * N_MBLK
        xr8_sb = [None] * N_MBLK

        def load_x_blk(blk):
            t = consts.tile([128, KC, M_BLK], FP8, name=f"x8{blk}", tag=f"x8{blk}")
            nc.sync.dma_start(
                t[:],
                x8_d.ap()[:, blk * M_BLK : (blk + 1) * M_BLK].rearrange(
                    "(kc p) m -> p kc m", p=128
                ),
            )
            x8_sb[blk] = t

        def load_xr_blk(blk):
            t = consts.tile([128, KC, M_BLK], FP8, name=f"xr8{blk}", tag=f"xr8{blk}")
            nc.sync.dma_start(
                t[:],
                xr8_d.ap()[:, blk * M_BLK : (blk + 1) * M_BLK].rearrange(
                    "(kc p) m -> p kc m", p=128
                ),
            )
            xr8_sb[blk] = t

        # PE pre-warm: dummy matmuls on memset tiles run during the initial
        # weight DMA so the p-state ramp burns idle time, not real work.
        warm_w = consts.tile([128, 128], BF16, name="warm_w", tag="warm_w")
        warm_x = consts.tile([128, M_BLK], BF16, name="warm_x", tag="warm_x")
        nc.vector.memset(warm_w[:], 0.0)
        nc.vector.memset(warm_x[:], 0.0)
        for _ in range(16):
            ph = psum_h.tile([128, M_BLK], F32, name="ph", tag="ph")
            nc.tensor.matmul(ph[:], lhsT=warm_w[:], rhs=warm_x[:])

        w1_sb = {n: [] for n in ("o", "f")}
        wr1_sb = {n: [] for n in ("o", "f")}

        def load_w_group(n, g, dram, store):
            t = consts.tile(
                [128, KC, GS], FP8, name=f"w{n}g{g}_{len(store[n])}", tag=f"w{n}g{g}_{id(dram)}"
            )
            nc.sync.dma_start(
                t[:],
                dram.ap()[:, g * GS : (g + 1) * GS].rearrange(
                    "(kc p) d -> p kc d", p=128
                ),
            )
            store[n].append(t)

        # consumption order: x8 b0, per-group (W8o, W8f, Wr8f), xr8 b0, rest
        load_x_blk(0)
        load_w_group("o", 0, w1_d["o"], w1_sb)
        load_w_group("f", 0, w1_d["f"], w1_sb)
        if CFG["f"][0] > 0:
            load_w_group("f", 0, wr1_d["f"], wr1_sb)
        if need_xr:
            load_xr_blk(0)

        w2_sb = {}
        b1_sb = {}
        b2_sb = {}
        for n in ("o", "f"):
            t = consts.tile([128, DC, C], BF16, name=f"w2{n}", tag=f"w2{n}")
            nc.sync.dma_start(
                t[:], w2_d[n].ap().rearrange("(dc p) c -> p dc c", p=128)
            )
            w2_sb[n] = t
            t = consts.tile([128, DC], F32, name=f"b1{n}", tag=f"b1{n}")
            nc.sync.dma_start(t[:], b1_d[n].ap().rearrange("(dc p) -> p dc", p=128))
            b1_sb[n] = t
            t = consts.tile([128, C], F32, name=f"b2{n}", tag=f"b2{n}")
            nc.sync.dma_start(
                t[:],
                bass.AP(tensor=b2_d[n], offset=0, ap=[[0, 128], [1, C]]),
            )
            b2_sb[n] = t

        for g in range(1, NG):
            load_w_group("o", g, w1_d["o"], w1_sb)
            load_w_group("f", g, w1_d["f"], w1_sb)
            if CFG["f"][0] > 0:
                load_w_group("f", g, wr1_d["f"], wr1_sb)
        if CFG["o"][0] > 0:
            for g in range(NG):
                load_w_group("o", g, wr1_d["o"], wr1_sb)

        for blk in range(1, N_MBLK):
            load_x_blk(blk)
            if need_xr:
                load_xr_blk(blk)

        NMH = M_BLK // 256  # 2 moving halves (DoubleRow rhs free limit)

        def layer1_dr(n, blk, dc, ph):
            """fp8 DoubleRow matmuls for one [128 dh x M_BLK] psum tile."""
            cw, cx = CFG[n]
            g, dl = dc // DC_G, dc % DC_G
            dsl = slice(dl * 128, (dl + 1) * 128)
            w8 = w1_sb[n][g]
            terms = [(w8, x8_sb[blk])]
            if cw > 0:
                terms.append((wr1_sb[n][g], x8_sb[blk]))
            if cx > 0:
                terms.append((w8, xr8_sb[blk]))
            for mh in range(NMH):
                msl = slice(mh * 256, (mh + 1) * 256)
                first, last = None, None
                seq = []
                for ti, (wt, xt) in enumerate(terms):
                    climit = KC if ti == 0 else (cw if ti == 1 and cw > 0 else cx)
                    # corrections apply to the FIRST climit chunks
                    for j in range(0, climit, 2):
                        seq.append((wt, xt, j))
                for i, (wt, xt, j) in enumerate(seq):
                    nc.tensor.matmul(
                        ph[:, msl],
                        lhsT=wt[:, j : j + 2, dsl],
                        rhs=xt[:, j : j + 2, msl],
                        start=(i == 0),
                        stop=(i == len(seq) - 1),
                        perf_mode=DR,
                    )

        # ---- main loop ---------------------------------------------------
        for blk in range(N_MBLK * reps):
            blk = blk % N_MBLK
            m0 = blk * M_BLK
            probs = {}
            for n in ("o", "f"):
                po = [
                    psum_o.tile([128, C], F32, name=f"po{mm}", tag=f"po{mm}")
                    for mm in range(MM)
                ]
                for dc in range(DC):
                    ph = psum_h.tile([128, M_BLK], F32, name="ph", tag="ph")
                    layer1_dr(n, blk, dc, ph)
                    hT = hpool.tile([128, M_BLK], BF16, name="hT", tag="hT")
                    nc.scalar.activation(
                        hT[:],
                        ph[:],
                        Act.Relu,
                        bias=b1_sb[n][:, dc : dc + 1],
                        scale=1.0 / W_SCALE,
                    )
                    for mm in range(MM):
                        nc.tensor.matmul(
                            po[mm][:],
                            lhsT=hT[:, mm * 128 : (mm + 1) * 128],
                            rhs=w2_sb[n][:, dc, :],
                            start=(dc == 0),
                            stop=(dc == DC - 1),
                        )
                # softmax over C, batched [128, MM, C]
                z = epool.tile([128, MM, C], F32, name=f"z{n}", tag=f"z{n}")
                negmax = epool.tile([128, MM], F32, name=f"ngm{n}", tag=f"ngm{n}")
                exps = epool.tile([128, MM, C], F32, name=f"ex{n}", tag=f"ex{n}")
                sums = epool.tile([128, MM], F32, name=f"sm{n}", tag=f"sm{n}")
                rinv = epool.tile([128, MM], F32, name=f"ri{n}", tag=f"ri{n}")
                wrk = epool.tile([128, MM], F32, name=f"wk{n}", tag=f"wk{n}")
                for mm in range(MM):
                    nc.vector.tensor_tensor(
                        z[:, mm, :], po[mm][:], b2_sb[n][:], Alu.add
                    )
                for mm in range(MM):
                    nc.vector.tensor_reduce(
                        negmax[:, mm : mm + 1],
                        z[:, mm, :],
                        axis=mybir.AxisListType.X,
                        op=Alu.max,
                        negate=True,
                    )
                for mm in range(MM):
                    nc.scalar.activation(
                        exps[:, mm, :],
                        z[:, mm, :],
                        Act.Exp,
                        bias=negmax[:, mm : mm + 1],
                        accum_out=sums[:, mm : mm + 1],
                    )
                nc.vector.reciprocal(rinv[:], sums[:])
                # one Newton step: r <- r * (2 - s*r)
                nc.vector.tensor_tensor(wrk[:], sums[:], rinv[:], Alu.mult)
                nc.vector.tensor_scalar(wrk[:], wrk[:], -1.0, 2.0, Alu.mult, Alu.add)
                nc.vector.tensor_tensor(rinv[:], rinv[:], wrk[:], Alu.mult)
                pr = epool.tile([128, MM, C], F32, name=f"pr{n}", tag=f"pr{n}")
                nc.vector.tensor_tensor(
                    pr[:],
                    exps[:],
                    rinv[:, :, None].to_broadcast([128, MM, C]),
                    Alu.mult,
                )
                probs[n] = pr

            # ---- comp_max_tau on probs["o"] ------------------------------
            res = epool.tile([128, MM, C + 1], F32, name="res", tag="res")
            s4 = epool.tile([128, MM], F32, name="s4", tag="s4")
            u4 = epool.tile([128, MM], F32, name="u4", tag="u4")
            b4 = epool.tile([128, MM], F32, name="b4", tag="b4")
            a4 = epool.tile([128, MM], F32, name="a4", tag="a4")
            nc.vector.tensor_scalar(
                res[:, :, 0:C], probs["o"][:], 0.0, None, Alu.add
            )
            nc.vector.memset(res[:, :, C : C + 1], TAU)
            for i in range(T1):
                m_i = 2.0 + TAU * TAU if i == 0 else 2.0
                k_i = 2.0 / m_i
                nc.vector.tensor_tensor(res[:], res[:], res[:], Alu.mult)
                nc.vector.tensor_reduce(
                    s4[:], res[:], axis=mybir.AxisListType.X, op=Alu.add
                )
                nc.vector.tensor_scalar(u4[:], s4[:], k_i, None, Alu.mult)
                nc.vector.tensor_scalar(b4[:], u4[:], -1.0, 1.0, Alu.mult, Alu.add)
                nc.vector.tensor_scalar(
                    a4[:], u4[:], -k_i, 2.0 * k_i, Alu.mult, Alu.add
                )
                for _ in range(T2):
                    nc.vector.tensor_tensor(b4[:], b4[:], b4[:], Alu.mult)
                    nc.vector.scalar_tensor_tensor(
                        a4[:], b4[:], 1.0, a4[:], Alu.add, Alu.mult
                    )
                nc.vector.tensor_tensor(
                    res[:],
                    res[:],
                    a4[:, :, None].to_broadcast([128, MM, C + 1]),
                    Alu.mult,
                )

            # ---- blend: out = x1 + cond * (x2 - x1) ----------------------
            dd = epool.tile([128, MM, C], F32, name="dd", tag="dd")
            outt = opool.tile([128, MM, C], F32, name="outt", tag="outt")
            nc.vector.tensor_tensor(dd[:], probs["f"][:], probs["o"][:], Alu.subtract)
            nc.vector.tensor_tensor(
                dd[:],
                dd[:],
                res[:, :, C : C + 1].to_broadcast([128, MM, C]),
                Alu.mult,
            )
            nc.vector.tensor_tensor(outt[:], dd[:], probs["o"][:], Alu.add)
            nc.sync.dma_start(
                out_d.ap()[m0 : m0 + M_BLK, :].rearrange("(mm p) c -> p mm c", p=128),
                outt[:],
            )

    nc.compile()
    return nc


def _get_module():
    if "nc" not in _BUILD_CACHE:
        _BUILD_CACHE["nc"] = _build_module()
    return _BUILD_CACHE["nc"]


def kernel(x, W1o, b1o, W2o, b2o, W1f, b1f, W2f, b2f):
    x = np.asarray(x, dtype=np.float32)
    bf = ml_dtypes.bfloat16
    e4 = ml_dtypes.float8_e4m3

    x8 = x.astype(e4)
    need_xr = any(cfg[1] > 0 for cfg in CFG.values())
    xr8 = (x - x8.astype(np.float32)).astype(e4) if need_xr else None

    w1q = {}
    wr1q = {}
    for n, W1 in (("o", W1o), ("f", W1f)):
        Ws = np.asarray(W1, np.float32) * W_SCALE
        W8 = Ws.astype(e4)
        w1q[n] = np.ascontiguousarray(W8)
        if CFG[n][0] > 0:
            wr1q[n] = np.ascontiguousarray((Ws - W8.astype(np.float32)).astype(e4))
    w2 = {
        "o": np.ascontiguousarray(np.asarray(W2o, np.float32).astype(bf)),
        "f": np.ascontiguousarray(np.asarray(W2f, np.float32).astype(bf)),
    }
    b1 = {
        "o": np.ascontiguousarray(np.asarray(b1o, np.float32)),
        "f": np.ascontiguousarray(np.asarray(b1f, np.float32)),
    }
    b2 = {
        "o": np.ascontiguousarray(np.asarray(b2o, np.float32)),
        "f": np.ascontiguousarray(np.asarray(b2f, np.float32)),
    }

    nc = _get_module()

    in_maps = []
    for i in range(N_CORES):
        sl = slice(i * M, (i + 1) * M)
        m = {"x8T": np.ascontiguousarray(x8[sl].T)}
        if need_xr:
            m["xr8T"] = np.ascontiguousarray(xr8[sl].T)
        for n in ("o", "f"):
            m[f"w1{n}"] = w1q[n]
            if CFG[n][0] > 0:
                m[f"wr1{n}"] = wr1q[n]
            m[f"w2{n}"] = w2[n]
            m[f"b1{n}"] = b1[n]
            m[f"b2{n}"] = b2[n]
        in_maps.append(m)

    trace = bool(os.environ.get("KERNEL_TRACE"))
    results = run_bass_kernel_spmd(
        nc, in_maps, list(range(N_CORES)), trace=trace
    )
    global LAST_RESULTS
    LAST_RESULTS = results

    out = np.concatenate(
        [np.asarray(results.results[i]["out"], np.float32) for i in range(N_CORES)],
        axis=0,
    )
    return out


# revision 13
# speedup vs baseline: 1.7109x; 1.0822x over previous
"""Trainium2 Bass kernel for nn_CombNetHE (dual-MLP + HE-friendly soft blend).

Contract: kernel(**inputs) takes FULL unsharded fp32 inputs (numpy or jax
arrays) and returns the FULL [16384, 10] fp32 output. Internally shards the
batch across 8 NeuronCores (2048 rows each), runs a fused Tile kernel per
core, and gathers on the host.

Layer 1 runs on the PE in fp8-e4m3 with DoubleRow perf mode (2 stationary
planes per instruction, 0.5 cycles per output element -> 4x bf16 FLOP rate).
Accuracy is managed asymmetrically, informed by an offline error study:
  - net_orig (only feeds the soft cond indicator + the mostly-suppressed
    (1-cond) blend arm) runs pure fp8: h = x8 @ W8.
  - net_fake (dominates the output since cond~1 for most rows) gets two
    fp8 correction products per k-chunk: the weight-quantization residual
    (x8 @ e4m3(Ws - W8)) and the x-quantization residual (e4m3(x - x8) @ W8),
    recovering ~bf16-level h accuracy at fp8 speed.
W1 is pre-scaled by 32 on the host (std 1/32 -> ~1) to sit in e4m3's normal
range; the 1/32 descale folds into the ReLU activation's scale operand.

Per-core layout:
  - x is pre-transposed/quantized on host to x8T/xr8T [D_IN, M] e4m3 so layer
    1 runs with W1 stationary, producing h transposed (hT[dh, m]) in PSUM;
    hT (bf16) is then directly the stationary operand of layer 2.
  - relu(h/32 + b1) + bf16 cast is a single ScalarE activation per [128, 512]
    tile, PSUM -> SBUF.
  - softmax / comp_max_tau / blend run in fp32 on DVE+ACT, batched over the
    whole 512-row block ([128 partitions x 4 chunks]).
"""

import os
import sys

for _p in ("/opt/trn_rl_repo", "/root/.axon_site/_ro/trn_rl_repo"):
    if os.path.isdir(_p) and _p not in sys.path:
        sys.path.insert(0, _p)

from contextlib import ExitStack

import ml_dtypes
import numpy as np

import concourse.bass as bass
import concourse.bacc as bacc
import concourse.mybir as mybir
import concourse.tile as tile
from concourse.bass_utils import run_bass_kernel_spmd

B, D_IN, D_H, C = 16384, 1024, 4096, 10
TAU, T1, T2 = 0.5, 3, 3
N_CORES = 8
M = B // N_CORES  # rows per core
M_BLK = 512  # rows processed per outer block
N_MBLK = M // M_BLK  # 4
MM = M_BLK // 128  # 4 partition-chunks per block
KC = D_IN // 128  # 8 contraction chunks (layer 1)
DC = D_H // 128  # 32 hidden chunks
W_SCALE = 32.0  # host pre-scale on W1 before e4m3 quantization

# fp8 correction products per net: (cw, cx) = # of k-chunks getting the
# W-residual / x-residual correction matmuls (0..KC each).
CFG = {"o": (0, 0), "f": (KC, KC)}

F32 = mybir.dt.float32
BF16 = mybir.dt.bfloat16
FP8 = mybir.dt.float8e4
Alu = mybir.AluOpType
Act = mybir.ActivationFunctionType
DR = mybir.MatmulPerfMode.DoubleRow

LAST_RESULTS = None
_BUILD_CACHE = {}


def _build_module(reps=1):
    nc = bacc.Bacc(
        "TRN2", target_bir_lowering=False, debug=False, num_devices=N_CORES
    )

    x8_d = nc.dram_tensor("x8T", [D_IN, M], FP8, kind="ExternalInput")
    need_xr = any(cfg[1] > 0 for cfg in CFG.values())
    xr8_d = (
        nc.dram_tensor("xr8T", [D_IN, M], FP8, kind="ExternalInput")
        if need_xr
        else None
    )
    w1_d = {}
    wr1_d = {}
    w2_d = {}
    b1_d = {}
    b2_d = {}
    for n in ("o", "f"):
        w1_d[n] = nc.dram_tensor(f"w1{n}", [D_IN, D_H], FP8, kind="ExternalInput")
        if CFG[n][0] > 0:
            wr1_d[n] = nc.dram_tensor(
                f"wr1{n}", [D_IN, D_H], FP8, kind="ExternalInput"
            )
        w2_d[n] = nc.dram_tensor(f"w2{n}", [D_H, C], BF16, kind="ExternalInput")
        b1_d[n] = nc.dram_tensor(f"b1{n}", [D_H], F32, kind="ExternalInput")
        b2_d[n] = nc.dram_tensor(f"b2{n}", [C], F32, kind="ExternalInput")
    out_d = nc.dram_tensor("out", [M, C], F32, kind="ExternalOutput")

    with tile.TileContext(nc) as tc, ExitStack() as ctx:
        consts = ctx.enter_context(tc.tile_pool(name="consts", bufs=1))
        hpool = ctx.enter_context(tc.tile_pool(name="hpool", bufs=8))
        rpool = ctx.enter_context(tc.tile_pool(name="rpool", bufs=3))
        epool = ctx.enter_context(tc.tile_pool(name="epool", bufs=3))
        opool = ctx.enter_context(tc.tile_pool(name="opool", bufs=3))
        psum_h = ctx.enter_context(tc.tile_pool(name="psum_h", bufs=4, space="PSUM"))
        psum_o = ctx.enter_context(tc.tile_pool(name="psum_o", bufs=1, space="PSUM"))

        # ---- resident tensors -------------------------------------------
        # DMAs are emitted in consumption order so PE can start early.
        GS = 512  # dh elements per weight group tile
        NG = D_H // GS  # 8 groups
        DC_G = GS // 128  # 4 dh chunks per group

        # x8/xr8: per-blk tiles [128, KC, M_BLK] fp8; [p, kc, m] = x[kc*128+p, m]
        x8_sb = [None] * N_MBLK
        xr8_sb = [None] * N_MBLK

        def load_x_blk(blk):
            t = consts.tile([128, KC, M_BLK], FP8, name=f"x8{blk}", tag=f"x8{blk}")
            nc.sync.dma_start(
                t[:],
                x8_d.ap()[:, blk * M_BLK : (blk + 1) * M_BLK].rearrange(
                    "(kc p) m -> p kc m", p=128
                ),
            )
            x8_sb[blk] = t

        def load_xr_blk(blk):
            t = consts.tile([128, KC, M_BLK], FP8, name=f"xr8{blk}", tag=f"xr8{blk}")
            nc.sync.dma_start(
                t[:],
                xr8_d.ap()[:, blk * M_BLK : (blk + 1) * M_BLK].rearrange(
                    "(kc p) m -> p kc m", p=128
                ),
            )
            xr8_sb[blk] = t

        # PE pre-warm: dummy matmuls on memset tiles run during the initial
        # weight DMA so the p-state ramp burns idle time, not real work.
        warm_w = consts.tile([128, 128], BF16, name="warm_w", tag="warm_w")
        warm_x = consts.tile([128, M_BLK], BF16, name="warm_x", tag="warm_x")
        nc.vector.memset(warm_w[:], 0.0)
        nc.vector.memset(warm_x[:], 0.0)
        for _ in range(10):
            ph = psum_h.tile([128, M_BLK], F32, name="ph", tag="ph")
            nc.tensor.matmul(ph[:], lhsT=warm_w[:], rhs=warm_x[:])

        w1_sb = {n: [] for n in ("o", "f")}
        wr1_sb = {n: [] for n in ("o", "f")}

        def load_w_group(n, g, dram, store):
            t = consts.tile(
                [128, KC, GS], FP8, name=f"w{n}g{g}_{len(store[n])}", tag=f"w{n}g{g}_{id(dram)}"
            )
            nc.sync.dma_start(
                t[:],
                dram.ap()[:, g * GS : (g + 1) * GS].rearrange(
                    "(kc p) d -> p kc d", p=128
                ),
            )
            store[n].append(t)

        # consumption order: o-phase of blk0 runs first, so feed it first:
        # x8 b0, all W8o groups (+b1o/w2o), then the f-phase set (W8f/Wr8f
        # groups, xr8 b0, b1f/w2f), then b2s and the remaining x blocks.
        w2_sb = {}
        b1_sb = {}
        b2_sb = {}

        def load_small(n):
            t = consts.tile([128, DC, C], BF16, name=f"w2{n}", tag=f"w2{n}")
            nc.sync.dma_start(
                t[:], w2_d[n].ap().rearrange("(dc p) c -> p dc c", p=128)
            )
            w2_sb[n] = t
            t = consts.tile([128, DC], F32, name=f"b1{n}", tag=f"b1{n}")
            nc.sync.dma_start(t[:], b1_d[n].ap().rearrange("(dc p) -> p dc", p=128))
            b1_sb[n] = t
            t = consts.tile([128, C], F32, name=f"b2{n}", tag=f"b2{n}")
            nc.sync.dma_start(
                t[:],
                bass.AP(tensor=b2_d[n], offset=0, ap=[[0, 128], [1, C]]),
            )
            b2_sb[n] = t

        load_x_blk(0)
        load_w_group("o", 0, w1_d["o"], w1_sb)
        load_small("o")
        for g in range(1, NG):
            load_w_group("o", g, w1_d["o"], w1_sb)
        if CFG["o"][0] > 0:
            for g in range(NG):
                load_w_group("o", g, wr1_d["o"], wr1_sb)

        load_w_group("f", 0, w1_d["f"], w1_sb)
        if CFG["f"][0] > 0:
            load_w_group("f", 0, wr1_d["f"], wr1_sb)
        if need_xr:
            load_xr_blk(0)
        load_small("f")
        for g in range(1, NG):
            load_w_group("f", g, w1_d["f"], w1_sb)
            if CFG["f"][0] > 0:
                load_w_group("f", g, wr1_d["f"], wr1_sb)

        for blk in range(1, N_MBLK):
            load_x_blk(blk)
            if need_xr:
                load_xr_blk(blk)

        NMH = M_BLK // 256  # 2 moving halves (DoubleRow rhs free limit)

        def layer1_dr(n, blk, dc, ph):
            """fp8 DoubleRow matmuls for one [128 dh x M_BLK] psum tile."""
            cw, cx = CFG[n]
            g, dl = dc // DC_G, dc % DC_G
            dsl = slice(dl * 128, (dl + 1) * 128)
            w8 = w1_sb[n][g]
            terms = [(w8, x8_sb[blk])]
            if cw > 0:
                terms.append((wr1_sb[n][g], x8_sb[blk]))
            if cx > 0:
                terms.append((w8, xr8_sb[blk]))
            for mh in range(NMH):
                msl = slice(mh * 256, (mh + 1) * 256)
                first, last = None, None
                seq = []
                for ti, (wt, xt) in enumerate(terms):
                    climit = KC if ti == 0 else (cw if ti == 1 and cw > 0 else cx)
                    # corrections apply to the FIRST climit chunks
                    for j in range(0, climit, 2):
                        seq.append((wt, xt, j))
                for i, (wt, xt, j) in enumerate(seq):
                    nc.tensor.matmul(
                        ph[:, msl],
                        lhsT=wt[:, j : j + 2, dsl],
                        rhs=xt[:, j : j + 2, msl],
                        start=(i == 0),
                        stop=(i == len(seq) - 1),
                        perf_mode=DR,
                    )

        # o-phase (PE ~440ns/dc) is ACT-bound at ~612ns/dc; route some of its
        # relu tiles to the mostly-idle DVE. The first 3 dc also hide the
        # ACT relu-table reload behind DVE work at each phase start.
        DVE_RELU_DCS = {0, 1, 2, 5, 8, 11, 14, 17, 20, 23}

        # ---- main loop ---------------------------------------------------
        for blk in range(N_MBLK * reps):
            blk = blk % N_MBLK
            m0 = blk * M_BLK
            last_blk = blk == N_MBLK - 1
            probs = {}
            # last block runs net f first so the closing serial chain
            # (softmax+comp+blend) is the short o-side one
            for n in ("f", "o") if last_blk else ("o", "f"):
                po = [
                    psum_o.tile([128, C], F32, name=f"po{mm}", tag=f"po{mm}")
                    for mm in range(MM)
                ]
                n_dr = 8 + sum(CFG[n])  # DR products per dc for this net
                for dc in range(DC):
                    ph = psum_h.tile([128, M_BLK], F32, name="ph", tag="ph")
                    layer1_dr(n, blk, dc, ph)
                    hT = hpool.tile([128, M_BLK], BF16, name="hT", tag="hT")
                    if n_dr <= 8 and dc in DVE_RELU_DCS:
                        t1 = rpool.tile([128, M_BLK], F32, name="t1", tag="t1")
                        nc.vector.scalar_tensor_tensor(
                            t1[:],
                            ph[:],
                            1.0 / W_SCALE,
                            b1_sb[n][:, dc : dc + 1].to_broadcast([128, M_BLK]),
                            Alu.mult,
                            Alu.add,
                        )
                        nc.vector.tensor_scalar(
                            hT[:], t1[:], 0.0, None, Alu.max
                        )
                    else:
                        nc.scalar.activation(
                            hT[:],
                            ph[:],
                            Act.Relu,
                            bias=b1_sb[n][:, dc : dc + 1],
                            scale=1.0 / W_SCALE,
                        )
                    for mm in range(MM):
                        nc.tensor.matmul(
                            po[mm][:],
                            lhsT=hT[:, mm * 128 : (mm + 1) * 128],
                            rhs=w2_sb[n][:, dc, :],
                            start=(dc == 0),
                            stop=(dc == DC - 1),
                        )
                # softmax over C, batched [128, MM, C]
                z = epool.tile([128, MM, C], F32, name=f"z{n}", tag=f"z{n}")
                negmax = epool.tile([128, MM], F32, name=f"ngm{n}", tag=f"ngm{n}")
                exps = epool.tile([128, MM, C], F32, name=f"ex{n}", tag=f"ex{n}")
                sums = epool.tile([128, MM], F32, name=f"sm{n}", tag=f"sm{n}")
                rinv = epool.tile([128, MM], F32, name=f"ri{n}", tag=f"ri{n}")
                wrk = epool.tile([128, MM], F32, name=f"wk{n}", tag=f"wk{n}")
                for mm in range(MM):
                    nc.vector.tensor_tensor(
                        z[:, mm, :], po[mm][:], b2_sb[n][:], Alu.add
                    )
                for mm in range(MM):
                    nc.vector.tensor_reduce(
                        negmax[:, mm : mm + 1],
                        z[:, mm, :],
                        axis=mybir.AxisListType.X,
                        op=Alu.max,
                        negate=True,
                    )
                for mm in range(MM):
                    nc.scalar.activation(
                        exps[:, mm, :],
                        z[:, mm, :],
                        Act.Exp,
                        bias=negmax[:, mm : mm + 1],
                        accum_out=sums[:, mm : mm + 1],
                    )
                nc.vector.reciprocal(rinv[:], sums[:])
                # one Newton step: r <- r * (2 - s*r)
                nc.vector.tensor_tensor(wrk[:], sums[:], rinv[:], Alu.mult)
                nc.vector.tensor_scalar(wrk[:], wrk[:], -1.0, 2.0, Alu.mult, Alu.add)
                nc.vector.tensor_tensor(rinv[:], rinv[:], wrk[:], Alu.mult)
                pr = epool.tile([128, MM, C], F32, name=f"pr{n}", tag=f"pr{n}")
                nc.vector.tensor_tensor(
                    pr[:],
                    exps[:],
                    rinv[:, :, None].to_broadcast([128, MM, C]),
                    Alu.mult,
                )
                probs[n] = pr

            # ---- comp_max_tau on probs["o"] ------------------------------
            res = epool.tile([128, MM, C + 1], F32, name="res", tag="res")
            s4 = epool.tile([128, MM], F32, name="s4", tag="s4")
            u4 = epool.tile([128, MM], F32, name="u4", tag="u4")
            b4 = epool.tile([128, MM], F32, name="b4", tag="b4")
            a4 = epool.tile([128, MM], F32, name="a4", tag="a4")
            nc.vector.tensor_scalar(
                res[:, :, 0:C], probs["o"][:], 0.0, None, Alu.add
            )
            nc.vector.memset(res[:, :, C : C + 1], TAU)
            for i in range(T1):
                m_i = 2.0 + TAU * TAU if i == 0 else 2.0
                k_i = 2.0 / m_i
                nc.vector.tensor_tensor(res[:], res[:], res[:], Alu.mult)
                nc.vector.tensor_reduce(
                    s4[:], res[:], axis=mybir.AxisListType.X, op=Alu.add
                )
                nc.vector.tensor_scalar(u4[:], s4[:], k_i, None, Alu.mult)
                nc.vector.tensor_scalar(b4[:], u4[:], -1.0, 1.0, Alu.mult, Alu.add)
                nc.vector.tensor_scalar(
                    a4[:], u4[:], -k_i, 2.0 * k_i, Alu.mult, Alu.add
                )
                for _ in range(T2):
                    nc.vector.tensor_tensor(b4[:], b4[:], b4[:], Alu.mult)
                    nc.vector.scalar_tensor_tensor(
                        a4[:], b4[:], 1.0, a4[:], Alu.add, Alu.mult
                    )
                nc.vector.tensor_tensor(
                    res[:],
                    res[:],
                    a4[:, :, None].to_broadcast([128, MM, C + 1]),
                    Alu.mult,
                )

            # ---- blend: out = x1 + cond * (x2 - x1) ----------------------
            dd = epool.tile([128, MM, C], F32, name="dd", tag="dd")
            outt = opool.tile([128, MM, C], F32, name="outt", tag="outt")
            nc.vector.tensor_tensor(dd[:], probs["f"][:], probs["o"][:], Alu.subtract)
            nc.vector.tensor_tensor(
                dd[:],
                dd[:],
                res[:, :, C : C + 1].to_broadcast([128, MM, C]),
                Alu.mult,
            )
            nc.vector.tensor_tensor(outt[:], dd[:], probs["o"][:], Alu.add)
            nc.sync.dma_start(
                out_d.ap()[m0 : m0 + M_BLK, :].rearrange("(mm p) c -> p mm c", p=128),
                outt[:],
            )

    nc.compile()
    return nc


def _get_module():
    if "nc" not in _BUILD_CACHE:
        _BUILD_CACHE["nc"] = _build_module()
    return _BUILD_CACHE["nc"]


def kernel(x, W1o, b1o, W2o, b2o, W1f, b1f, W2f, b2f):
    x = np.asarray(x, dtype=np.float32)
    bf = ml_dtypes.bfloat16
    e4 = ml_dtypes.float8_e4m3

    x8 = x.astype(e4)
    need_xr = any(cfg[1] > 0 for cfg in CFG.values())
    xr8 = (x - x8.astype(np.float32)).astype(e4) if need_xr else None

    w1q = {}
    wr1q = {}
    for n, W1 in (("o", W1o), ("f", W1f)):
        Ws = np.asarray(W1, np.float32) * W_SCALE
        W8 = Ws.astype(e4)
        w1q[n] = np.ascontiguousarray(W8)
        if CFG[n][0] > 0:
            wr1q[n] = np.ascontiguousarray((Ws - W8.astype(np.float32)).astype(e4))
    w2 = {
        "o": np.ascontiguousarray(np.asarray(W2o, np.float32).astype(bf)),
        "f": np.ascontiguousarray(np.asarray(W2f, np.float32).astype(bf)),
    }
    b1 = {
        "o": np.ascontiguousarray(np.asarray(b1o, np.float32)),
        "f": np.ascontiguousarray(np.asarray(b1f, np.float32)),
    }
    b2 = {
        "o": np.ascontiguousarray(np.asarray(b2o, np.float32)),
        "f": np.ascontiguousarray(np.asarray(b2f, np.float32)),
    }

    nc = _get_module()

    in_maps = []
    for i in range(N_CORES):
        sl = slice(i * M, (i + 1) * M)
        m = {"x8T": np.ascontiguousarray(x8[sl].T)}
        if need_xr:
            m["xr8T"] = np.ascontiguousarray(xr8[sl].T)
        for n in ("o", "f"):
            m[f"w1{n}"] = w1q[n]
            if CFG[n][0] > 0:
                m[f"wr1{n}"] = wr1q[n]
            m[f"w2{n}"] = w2[n]
            m[f"b1{n}"] = b1[n]
            m[f"b2{n}"] = b2[n]
        in_maps.append(m)

    trace = bool(os.environ.get("KERNEL_TRACE"))
    results = run_bass_kernel_spmd(
        nc, in_maps, list(range(N_CORES)), trace=trace
    )
    global LAST_RESULTS
    LAST_RESULTS = results

    out = np.concatenate(
        [np.asarray(results.results[i]["out"], np.float32) for i in range(N_CORES)],
        axis=0,
    )
    return out


# revision 17
# speedup vs baseline: 1.8965x; 1.1085x over previous
"""Trainium2 Bass kernel for nn_CombNetHE (dual-MLP + HE-friendly soft blend).

Contract: kernel(**inputs) takes FULL unsharded fp32 inputs (numpy or jax
arrays) and returns the FULL [16384, 10] fp32 output. Internally shards the
batch across 8 NeuronCores (2048 rows each), runs a fused Tile kernel per
core, and gathers on the host.

Layer 1 runs on the PE in fp8-e4m3 with DoubleRow perf mode (2 stationary
planes per instruction, 0.5 cycles per output element -> 4x bf16 FLOP rate).
Accuracy is managed asymmetrically, informed by an offline error study:
  - net_orig (only feeds the soft cond indicator + the mostly-suppressed
    (1-cond) blend arm) runs pure fp8: h = x8 @ W8.
  - net_fake (dominates the output since cond~1 for most rows) gets two
    fp8 correction products per k-chunk: the weight-quantization residual
    (x8 @ e4m3(Ws - W8)) and the x-quantization residual (e4m3(x - x8) @ W8),
    recovering ~bf16-level h accuracy at fp8 speed.
W1 is pre-scaled by 32 on the host (std 1/32 -> ~1) to sit in e4m3's normal
range; the 1/32 descale folds into the ReLU activation's scale operand.

Per-core layout:
  - x is pre-transposed/quantized on host to x8T/xr8T [D_IN, M] e4m3 so layer
    1 runs with W1 stationary, producing h transposed (hT[dh, m]) in PSUM;
    hT (bf16) is then directly the stationary operand of layer 2.
  - relu(h/32 + b1) + bf16 cast is a single ScalarE activation per [128, 512]
    tile, PSUM -> SBUF.
  - softmax / comp_max_tau / blend run in fp32 on DVE+ACT, batched over the
    whole 512-row block ([128 partitions x 4 chunks]).
"""

import os
import sys

for _p in ("/opt/trn_rl_repo", "/root/.axon_site/_ro/trn_rl_repo"):
    if os.path.isdir(_p) and _p not in sys.path:
        sys.path.insert(0, _p)

from contextlib import ExitStack

import ml_dtypes
import numpy as np

import concourse.bass as bass
import concourse.bacc as bacc
import concourse.mybir as mybir
import concourse.tile as tile
from concourse.bass_utils import run_bass_kernel_spmd

B, D_IN, D_H, C = 16384, 1024, 4096, 10
TAU, T1, T2 = 0.5, 3, 3
N_CORES = 8
M = B // N_CORES  # rows per core
M_BLK = 512  # rows processed per outer block
N_MBLK = M // M_BLK  # 4
MM = M_BLK // 128  # 4 partition-chunks per block
KC = D_IN // 128  # 8 contraction chunks (layer 1)
DC = D_H // 128  # 32 hidden chunks
W_SCALE = 32.0  # host pre-scale on W1 before e4m3 quantization

# fp8 correction products per net: (cw, cx) = # of k-chunks getting the
# W-residual / x-residual correction matmuls (0..KC each).
CFG = {"o": (0, 0), "f": (KC, KC)}

F32 = mybir.dt.float32
BF16 = mybir.dt.bfloat16
FP8 = mybir.dt.float8e4
Alu = mybir.AluOpType
Act = mybir.ActivationFunctionType
DR = mybir.MatmulPerfMode.DoubleRow

LAST_RESULTS = None
_BUILD_CACHE = {}


def _build_module(reps=1):
    nc = bacc.Bacc(
        "TRN2", target_bir_lowering=False, debug=False, num_devices=N_CORES
    )

    x8_d = nc.dram_tensor("x8T", [D_IN, M], FP8, kind="ExternalInput")
    need_xr = any(cfg[1] > 0 for cfg in CFG.values())
    xr8_d = (
        nc.dram_tensor("xr8T", [D_IN, M], FP8, kind="ExternalInput")
        if need_xr
        else None
    )
    w1_d = {}
    wr1_d = {}
    w2_d = {}
    b1_d = {}
    b2_d = {}
    for n in ("o", "f"):
        w1_d[n] = nc.dram_tensor(f"w1{n}", [D_IN, D_H], FP8, kind="ExternalInput")
        if CFG[n][0] > 0:
            wr1_d[n] = nc.dram_tensor(
                f"wr1{n}", [D_IN, D_H], FP8, kind="ExternalInput"
            )
        # host pre-repacked to partition-major layouts (cheap large-descriptor
        # DMAs instead of thousands of 4-20B gather descriptors)
        w2_d[n] = nc.dram_tensor(f"w2{n}", [128, D_H // 128, C], BF16, kind="ExternalInput")
        b1_d[n] = nc.dram_tensor(f"b1{n}", [128, D_H // 128], F32, kind="ExternalInput")
        b2_d[n] = nc.dram_tensor(f"b2{n}", [128, C], F32, kind="ExternalInput")
    out_d = nc.dram_tensor("out", [M, C], F32, kind="ExternalOutput")

    with tile.TileContext(nc) as tc, ExitStack() as ctx:
        consts = ctx.enter_context(tc.tile_pool(name="consts", bufs=1))
        hpool = ctx.enter_context(tc.tile_pool(name="hpool", bufs=8))
        rpool = ctx.enter_context(tc.tile_pool(name="rpool", bufs=3))
        epool = ctx.enter_context(tc.tile_pool(name="epool", bufs=3))
        opool = ctx.enter_context(tc.tile_pool(name="opool", bufs=3))
        psum_h = ctx.enter_context(tc.tile_pool(name="psum_h", bufs=4, space="PSUM"))
        psum_o = ctx.enter_context(tc.tile_pool(name="psum_o", bufs=1, space="PSUM"))

        # ---- resident tensors -------------------------------------------
        # DMAs are emitted in consumption order so PE can start early.
        GS = 512  # dh elements per weight group tile
        NG = D_H // GS  # 8 groups
        DC_G = GS // 128  # 4 dh chunks per group

        # x8/xr8: per-blk tiles [128, KC, M_BLK] fp8; [p, kc, m] = x[kc*128+p, m]
        x8_sb = [None] * N_MBLK
        xr8_sb = [None] * N_MBLK

        def load_x_blk(blk):
            t = consts.tile([128, KC, M_BLK], FP8, name=f"x8{blk}", tag=f"x8{blk}")
            nc.sync.dma_start(
                t[:],
                x8_d.ap()[:, blk * M_BLK : (blk + 1) * M_BLK].rearrange(
                    "(kc p) m -> p kc m", p=128
                ),
            )
            x8_sb[blk] = t

        def load_xr_blk(blk):
            t = consts.tile([128, KC, M_BLK], FP8, name=f"xr8{blk}", tag=f"xr8{blk}")
            nc.sync.dma_start(
                t[:],
                xr8_d.ap()[:, blk * M_BLK : (blk + 1) * M_BLK].rearrange(
                    "(kc p) m -> p kc m", p=128
                ),
            )
            xr8_sb[blk] = t

        # PE pre-warm: dummy matmuls on memset tiles run during the initial
        # weight DMA so the p-state ramp burns idle time, not real work.
        warm_w = consts.tile([128, 128], BF16, name="warm_w", tag="warm_w")
        warm_x = consts.tile([128, M_BLK], BF16, name="warm_x", tag="warm_x")
        nc.vector.memset(warm_w[:], 0.0)
        nc.vector.memset(warm_x[:], 0.0)
        for _ in range(10):
            ph = psum_h.tile([128, M_BLK], F32, name="ph", tag="ph")
            nc.tensor.matmul(ph[:], lhsT=warm_w[:], rhs=warm_x[:])

        w1_sb = {n: [] for n in ("o", "f")}
        wr1_sb = {n: [] for n in ("o", "f")}

        def load_w_group(n, g, dram, store):
            t = consts.tile(
                [128, KC, GS], FP8, name=f"w{n}g{g}_{len(store[n])}", tag=f"w{n}g{g}_{id(dram)}"
            )
            nc.sync.dma_start(
                t[:],
                dram.ap()[:, g * GS : (g + 1) * GS].rearrange(
                    "(kc p) d -> p kc d", p=128
                ),
            )
            store[n].append(t)

        # consumption order: o-phase of blk0 runs first, so feed it first:
        # x8 b0, all W8o groups (+b1o/w2o), then the f-phase set (W8f/Wr8f
        # groups, xr8 b0, b1f/w2f), then b2s and the remaining x blocks.
        w2_sb = {}
        b1_sb = {}
        b2_sb = {}

        def load_small(n):
            t = consts.tile([128, DC, C], BF16, name=f"w2{n}", tag=f"w2{n}")
            nc.sync.dma_start(t[:], w2_d[n].ap())
            w2_sb[n] = t
            t = consts.tile([128, DC], F32, name=f"b1{n}", tag=f"b1{n}")
            nc.sync.dma_start(t[:], b1_d[n].ap())
            b1_sb[n] = t
            t = consts.tile([128, C], F32, name=f"b2{n}", tag=f"b2{n}")
            nc.sync.dma_start(t[:], b2_d[n].ap())
            b2_sb[n] = t

        load_x_blk(0)
        load_w_group("o", 0, w1_d["o"], w1_sb)
        load_small("o")
        for g in range(1, NG):
            load_w_group("o", g, w1_d["o"], w1_sb)
        if CFG["o"][0] > 0:
            for g in range(NG):
                load_w_group("o", g, wr1_d["o"], wr1_sb)

        load_w_group("f", 0, w1_d["f"], w1_sb)
        if CFG["f"][0] > 0:
            load_w_group("f", 0, wr1_d["f"], wr1_sb)
        if need_xr:
            load_xr_blk(0)
        load_small("f")
        for g in range(1, NG):
            load_w_group("f", g, w1_d["f"], w1_sb)
            if CFG["f"][0] > 0:
                load_w_group("f", g, wr1_d["f"], wr1_sb)

        for blk in range(1, N_MBLK):
            load_x_blk(blk)
            if need_xr:
                load_xr_blk(blk)

        NMH = M_BLK // 256  # 2 moving halves (DoubleRow rhs free limit)

        def layer1_dr(n, blk, dc, ph):
            """fp8 DoubleRow matmuls for one [128 dh x M_BLK] psum tile."""
            cw, cx = CFG[n]
            g, dl = dc // DC_G, dc % DC_G
            dsl = slice(dl * 128, (dl + 1) * 128)
            w8 = w1_sb[n][g]
            terms = [(w8, x8_sb[blk])]
            if cw > 0:
                terms.append((wr1_sb[n][g], x8_sb[blk]))
            if cx > 0:
                terms.append((w8, xr8_sb[blk]))
            for mh in range(NMH):
                msl = slice(mh * 256, (mh + 1) * 256)
                first, last = None, None
                seq = []
                for ti, (wt, xt) in enumerate(terms):
                    climit = KC if ti == 0 else (cw if ti == 1 and cw > 0 else cx)
                    # corrections apply to the FIRST climit chunks
                    for j in range(0, climit, 2):
                        seq.append((wt, xt, j))
                for i, (wt, xt, j) in enumerate(seq):
                    nc.tensor.matmul(
                        ph[:, msl],
                        lhsT=wt[:, j : j + 2, dsl],
                        rhs=xt[:, j : j + 2, msl],
                        start=(i == 0),
                        stop=(i == len(seq) - 1),
                        perf_mode=DR,
                    )

        # o-phase (PE ~440ns/dc) is ACT-bound at ~612ns/dc; route some of its
        # relu tiles to the mostly-idle DVE. The first 3 dc also hide the
        # ACT relu-table reload behind DVE work at each phase start.
        DVE_RELU_DCS = {0, 1, 2, 5, 8, 11, 14, 17, 20, 23}

        # ---- main loop ---------------------------------------------------
        for blk in range(N_MBLK * reps):
            blk = blk % N_MBLK
            m0 = blk * M_BLK
            last_blk = blk == N_MBLK - 1
            probs = {}
            # last block runs net f first so the closing serial chain
            # (softmax+comp+blend) is the short o-side one
            for n in ("f", "o") if last_blk else ("o", "f"):
                po = [
                    psum_o.tile([128, C], F32, name=f"po{mm}", tag=f"po{mm}")
                    for mm in range(MM)
                ]
                n_dr = 8 + sum(CFG[n])  # DR products per dc for this net
                dve_dcs = (
                    set(range(5)) if last_blk else DVE_RELU_DCS
                )  # last blk: keep DVE clear for the closing chain

                def emit_l2(dc, hT):
                    for mm in range(MM):
                        nc.tensor.matmul(
                            po[mm][:],
                            lhsT=hT[:, mm * 128 : (mm + 1) * 128],
                            rhs=w2_sb[n][:, dc, :],
                            start=(dc == 0),
                            stop=(dc == DC - 1),
                        )

                pend_l2 = None  # layer-2 is deferred one dc so hT is ready
                for dc in range(DC):
                    ph = psum_h.tile([128, M_BLK], F32, name="ph", tag="ph")
                    layer1_dr(n, blk, dc, ph)
                    hT = hpool.tile([128, M_BLK], BF16, name="hT", tag="hT")
                    if n_dr <= 8 and dc in dve_dcs:
                        t1 = rpool.tile([128, M_BLK], F32, name="t1", tag="t1")
                        nc.vector.scalar_tensor_tensor(
                            t1[:],
                            ph[:],
                            1.0 / W_SCALE,
                            b1_sb[n][:, dc : dc + 1].to_broadcast([128, M_BLK]),
                            Alu.mult,
                            Alu.add,
                        )
                        nc.vector.tensor_scalar(
                            hT[:], t1[:], 0.0, None, Alu.max
                        )
                    else:
                        nc.scalar.activation(
                            hT[:],
                            ph[:],
                            Act.Relu,
                            bias=b1_sb[n][:, dc : dc + 1],
                            scale=1.0 / W_SCALE,
                        )
                    if pend_l2 is not None:
                        emit_l2(*pend_l2)
                    pend_l2 = (dc, hT)
                emit_l2(*pend_l2)
                # softmax over C, batched [128, MM, C]
                z = epool.tile([128, MM, C], F32, name=f"z{n}", tag=f"z{n}")
                negmax = epool.tile([128, MM], F32, name=f"ngm{n}", tag=f"ngm{n}")
                exps = epool.tile([128, MM, C], F32, name=f"ex{n}", tag=f"ex{n}")
                sums = epool.tile([128, MM], F32, name=f"sm{n}", tag=f"sm{n}")
                rinv = epool.tile([128, MM], F32, name=f"ri{n}", tag=f"ri{n}")
                wrk = epool.tile([128, MM], F32, name=f"wk{n}", tag=f"wk{n}")
                for mm in range(MM):
                    nc.vector.tensor_tensor(
                        z[:, mm, :], po[mm][:], b2_sb[n][:], Alu.add
                    )
                for mm in range(MM):
                    nc.vector.tensor_reduce(
                        negmax[:, mm : mm + 1],
                        z[:, mm, :],
                        axis=mybir.AxisListType.X,
                        op=Alu.max,
                        negate=True,
                    )
                for mm in range(MM):
                    nc.scalar.activation(
                        exps[:, mm, :],
                        z[:, mm, :],
                        Act.Exp,
                        bias=negmax[:, mm : mm + 1],
                        accum_out=sums[:, mm : mm + 1],
                    )
                nc.vector.reciprocal(rinv[:], sums[:])
                # one Newton step: r <- r * (2 - s*r)
                nc.vector.tensor_tensor(wrk[:], sums[:], rinv[:], Alu.mult)
                nc.vector.tensor_scalar(wrk[:], wrk[:], -1.0, 2.0, Alu.mult, Alu.add)
                nc.vector.tensor_tensor(rinv[:], rinv[:], wrk[:], Alu.mult)
                pr = epool.tile([128, MM, C], F32, name=f"pr{n}", tag=f"pr{n}")
                nc.vector.tensor_tensor(
                    pr[:],
                    exps[:],
                    rinv[:, :, None].to_broadcast([128, MM, C]),
                    Alu.mult,
                )
                probs[n] = pr

            # ---- comp_max_tau on probs["o"] ------------------------------
            res = epool.tile([128, MM, C + 1], F32, name="res", tag="res")
            s4 = epool.tile([128, MM], F32, name="s4", tag="s4")
            u4 = epool.tile([128, MM], F32, name="u4", tag="u4")
            b4 = epool.tile([128, MM], F32, name="b4", tag="b4")
            a4 = epool.tile([128, MM], F32, name="a4", tag="a4")
            nc.vector.tensor_scalar(
                res[:, :, 0:C], probs["o"][:], 0.0, None, Alu.add
            )
            nc.vector.memset(res[:, :, C : C + 1], TAU)
            for i in range(T1):
                m_i = 2.0 + TAU * TAU if i == 0 else 2.0
                k_i = 2.0 / m_i
                nc.vector.tensor_tensor(res[:], res[:], res[:], Alu.mult)
                nc.vector.tensor_reduce(
                    s4[:], res[:], axis=mybir.AxisListType.X, op=Alu.add
                )
                nc.vector.tensor_scalar(u4[:], s4[:], k_i, None, Alu.mult)
                nc.vector.tensor_scalar(b4[:], u4[:], -1.0, 1.0, Alu.mult, Alu.add)
                nc.vector.tensor_scalar(
                    a4[:], u4[:], -k_i, 2.0 * k_i, Alu.mult, Alu.add
                )
                for _ in range(T2):
                    nc.vector.tensor_tensor(b4[:], b4[:], b4[:], Alu.mult)
                    nc.vector.scalar_tensor_tensor(
                        a4[:], b4[:], 1.0, a4[:], Alu.add, Alu.mult
                    )
                nc.vector.tensor_tensor(
                    res[:],
                    res[:],
                    a4[:, :, None].to_broadcast([128, MM, C + 1]),
                    Alu.mult,
                )

            # ---- blend: out = x1 + cond * (x2 - x1) ----------------------
            dd = epool.tile([128, MM, C], F32, name="dd", tag="dd")
            outt = opool.tile([128, MM, C], F32, name="outt", tag="outt")
            nc.vector.tensor_tensor(dd[:], probs["f"][:], probs["o"][:], Alu.subtract)
            nc.vector.tensor_tensor(
                dd[:],
                dd[:],
                res[:, :, C : C + 1].to_broadcast([128, MM, C]),
                Alu.mult,
            )
            nc.vector.tensor_tensor(outt[:], dd[:], probs["o"][:], Alu.add)
            nc.sync.dma_start(
                out_d.ap()[m0 : m0 + M_BLK, :].rearrange("(mm p) c -> p mm c", p=128),
                outt[:],
            )

    nc.compile()
    return nc


def _get_module():
    if "nc" not in _BUILD_CACHE:
        _BUILD_CACHE["nc"] = _build_module()
    return _BUILD_CACHE["nc"]


def kernel(x, W1o, b1o, W2o, b2o, W1f, b1f, W2f, b2f):
    x = np.asarray(x, dtype=np.float32)
    bf = ml_dtypes.bfloat16
    e4 = ml_dtypes.float8_e4m3

    x8 = x.astype(e4)
    need_xr = any(cfg[1] > 0 for cfg in CFG.values())
    xr8 = (x - x8.astype(np.float32)).astype(e4) if need_xr else None

    w1q = {}
    wr1q = {}
    for n, W1 in (("o", W1o), ("f", W1f)):
        Ws = np.asarray(W1, np.float32) * W_SCALE
        W8 = Ws.astype(e4)
        w1q[n] = np.ascontiguousarray(W8)
        if CFG[n][0] > 0:
            wr1q[n] = np.ascontiguousarray((Ws - W8.astype(np.float32)).astype(e4))
    def pack_w2(W2):
        a = np.asarray(W2, np.float32).astype(bf).reshape(DC, 128, C)
        return np.ascontiguousarray(a.transpose(1, 0, 2))  # [128, DC, C]

    def pack_b1(b):
        a = np.asarray(b, np.float32).reshape(DC, 128)
        return np.ascontiguousarray(a.T)  # [128, DC]

    def pack_b2(b):
        return np.ascontiguousarray(
            np.broadcast_to(np.asarray(b, np.float32), (128, C))
        )

    w2 = {"o": pack_w2(W2o), "f": pack_w2(W2f)}
    b1 = {"o": pack_b1(b1o), "f": pack_b1(b1f)}
    b2 = {"o": pack_b2(b2o), "f": pack_b2(b2f)}

    nc = _get_module()

    in_maps = []
    for i in range(N_CORES):
        sl = slice(i * M, (i + 1) * M)
        m = {"x8T": np.ascontiguousarray(x8[sl].T)}
        if need_xr:
            m["xr8T"] = np.ascontiguousarray(xr8[sl].T)
        for n in ("o", "f"):
            m[f"w1{n}"] = w1q[n]
            if CFG[n][0] > 0:
                m[f"wr1{n}"] = wr1q[n]
            m[f"w2{n}"] = w2[n]
            m[f"b1{n}"] = b1[n]
            m[f"b2{n}"] = b2[n]
        in_maps.append(m)

    trace = bool(os.environ.get("KERNEL_TRACE"))
    results = run_bass_kernel_spmd(
        nc, in_maps, list(range(N_CORES)), trace=trace
    )
    global LAST_RESULTS
    LAST_RESULTS = results

    out = np.concatenate(
        [np.asarray(results.results[i]["out"], np.float32) for i in range(N_CORES)],
        axis=0,
    )
    return out


# revision 21
# speedup vs baseline: 2.2053x; 1.1628x over previous
"""Trainium2 Bass kernel for nn_CombNetHE (dual-MLP + HE-friendly soft blend).

Contract: kernel(**inputs) takes FULL unsharded fp32 inputs (numpy or jax
arrays) and returns the FULL [16384, 10] fp32 output. Internally shards the
batch across 8 NeuronCores (2048 rows each), runs a fused Tile kernel per
core, and gathers on the host.

Layer 1 runs on the PE in fp8-e4m3 with DoubleRow perf mode (2 stationary
planes per instruction, 0.5 cycles per output element -> 4x bf16 FLOP rate).
Accuracy is managed asymmetrically, informed by an offline error study:
  - net_orig (only feeds the soft cond indicator + the mostly-suppressed
    (1-cond) blend arm) runs pure fp8: h = x8 @ W8.
  - net_fake (dominates the output since cond~1 for most rows) gets two
    fp8 correction products per k-chunk: the weight-quantization residual
    (x8 @ e4m3(Ws - W8)) and the x-quantization residual (e4m3(x - x8) @ W8),
    recovering ~bf16-level h accuracy at fp8 speed.
W1 is pre-scaled by 32 on the host (std 1/32 -> ~1) to sit in e4m3's normal
range; the 1/32 descale folds into the ReLU activation's scale operand.

Per-core layout:
  - x is pre-transposed/quantized on host to x8T/xr8T [D_IN, M] e4m3 so layer
    1 runs with W1 stationary, producing h transposed (hT[dh, m]) in PSUM;
    hT (bf16) is then directly the stationary operand of layer 2.
  - relu(h/32 + b1) + bf16 cast is a single ScalarE activation per [128, 512]
    tile, PSUM -> SBUF.
  - softmax / comp_max_tau / blend run in fp32 on DVE+ACT, batched over the
    whole 512-row block ([128 partitions x 4 chunks]).
"""

import os
import sys

for _p in ("/opt/trn_rl_repo", "/root/.axon_site/_ro/trn_rl_repo"):
    if os.path.isdir(_p) and _p not in sys.path:
        sys.path.insert(0, _p)

from contextlib import ExitStack

import ml_dtypes
import numpy as np

import concourse.bass as bass
import concourse.bacc as bacc
import concourse.mybir as mybir
import concourse.tile as tile
from concourse.bass_utils import run_bass_kernel_spmd

B, D_IN, D_H, C = 16384, 1024, 4096, 10
TAU, T1, T2 = 0.5, 3, 3
N_CORES = 8
M = B // N_CORES  # rows per core
M_BLK = 512  # rows processed per outer block
N_MBLK = M // M_BLK  # 4
MM = M_BLK // 128  # 4 partition-chunks per block
KC = D_IN // 128  # 8 contraction chunks (layer 1)
DC = D_H // 128  # 32 hidden chunks
W_SCALE = 32.0  # host pre-scale on W1 before e4m3 quantization

# fp8 correction products per net: (cw, cx) = # of k-chunks getting the
# W-residual / x-residual correction matmuls (0..KC each, even). Chosen via
# offline error sweep: net f dominates the output (cond~1), so it gets the
# corrections; measured HW rel err tracks the numpy sim to <1% relative.
CFG = {"o": (0, 0), "f": (KC, 4)}
CXM = max(cfg[1] for cfg in CFG.values())  # xr8 chunks actually consumed

F32 = mybir.dt.float32
BF16 = mybir.dt.bfloat16
FP8 = mybir.dt.float8e4
Alu = mybir.AluOpType
Act = mybir.ActivationFunctionType
DR = mybir.MatmulPerfMode.DoubleRow

LAST_RESULTS = None
_BUILD_CACHE = {}


def _build_module(reps=1):
    nc = bacc.Bacc(
        "TRN2", target_bir_lowering=False, debug=False, num_devices=N_CORES
    )

    x8_d = nc.dram_tensor("x8T", [D_IN, M], FP8, kind="ExternalInput")
    need_xr = any(cfg[1] > 0 for cfg in CFG.values())
    xr8_d = (
        nc.dram_tensor("xr8T", [D_IN, M], FP8, kind="ExternalInput")
        if need_xr
        else None
    )
    w1_d = {}
    wr1_d = {}
    w2_d = {}
    b1_d = {}
    b2_d = {}
    for n in ("o", "f"):
        w1_d[n] = nc.dram_tensor(f"w1{n}", [D_IN, D_H], FP8, kind="ExternalInput")
        if CFG[n][0] > 0:
            wr1_d[n] = nc.dram_tensor(
                f"wr1{n}", [D_IN, D_H], FP8, kind="ExternalInput"
            )
        # host pre-repacked to partition-major layouts (cheap large-descriptor
        # DMAs instead of thousands of 4-20B gather descriptors)
        w2_d[n] = nc.dram_tensor(f"w2{n}", [128, D_H // 128, C], BF16, kind="ExternalInput")
        b1_d[n] = nc.dram_tensor(f"b1{n}", [128, D_H // 128], F32, kind="ExternalInput")
        b2_d[n] = nc.dram_tensor(f"b2{n}", [128, C], F32, kind="ExternalInput")
    out_d = nc.dram_tensor("out", [M, C], F32, kind="ExternalOutput")

    with tile.TileContext(nc) as tc, ExitStack() as ctx:
        consts = ctx.enter_context(tc.tile_pool(name="consts", bufs=1))
        hpool = ctx.enter_context(tc.tile_pool(name="hpool", bufs=8))
        rpool = ctx.enter_context(tc.tile_pool(name="rpool", bufs=3))
        epool = ctx.enter_context(tc.tile_pool(name="epool", bufs=3))
        opool = ctx.enter_context(tc.tile_pool(name="opool", bufs=3))
        psum_h = ctx.enter_context(tc.tile_pool(name="psum_h", bufs=4, space="PSUM"))
        psum_o = ctx.enter_context(tc.tile_pool(name="psum_o", bufs=1, space="PSUM"))

        # ---- resident tensors -------------------------------------------
        # DMAs are emitted in consumption order so PE can start early.
        GS = 512  # dh elements per weight group tile
        NG = D_H // GS  # 8 groups
        DC_G = GS // 128  # 4 dh chunks per group

        # x8/xr8: per-blk tiles [128, KC, M_BLK] fp8; [p, kc, m] = x[kc*128+p, m]
        x8_sb = [None] * N_MBLK
        xr8_sb = [None] * N_MBLK

        def load_x_blk(blk):
            t = consts.tile([128, KC, M_BLK], FP8, name=f"x8{blk}", tag=f"x8{blk}")
            nc.sync.dma_start(
                t[:],
                x8_d.ap()[:, blk * M_BLK : (blk + 1) * M_BLK].rearrange(
                    "(kc p) m -> p kc m", p=128
                ),
            )
            x8_sb[blk] = t

        def load_xr_blk(blk):
            # only the first CXM k-chunks are consumed by corrections
            t = consts.tile([128, CXM, M_BLK], FP8, name=f"xr8{blk}", tag=f"xr8{blk}")
            nc.sync.dma_start(
                t[:],
                xr8_d.ap()[
                    : CXM * 128, blk * M_BLK : (blk + 1) * M_BLK
                ].rearrange("(kc p) m -> p kc m", p=128),
            )
            xr8_sb[blk] = t

        # PE pre-warm: dummy matmuls on memset tiles run during the initial
        # weight DMA so the p-state ramp burns idle time, not real work.
        warm_w = consts.tile([128, 128], BF16, name="warm_w", tag="warm_w")
        warm_x = consts.tile([128, M_BLK], BF16, name="warm_x", tag="warm_x")
        nc.vector.memset(warm_w[:], 0.0)
        nc.vector.memset(warm_x[:], 0.0)
        for _ in range(10):
            ph = psum_h.tile([128, M_BLK], F32, name="ph", tag="ph")
            nc.tensor.matmul(ph[:], lhsT=warm_w[:], rhs=warm_x[:])

        w1_sb = {n: [] for n in ("o", "f")}
        wr1_sb = {n: [] for n in ("o", "f")}

        def load_w_group(n, g, dram, store):
            t = consts.tile(
                [128, KC, GS], FP8, name=f"w{n}g{g}_{len(store[n])}", tag=f"w{n}g{g}_{id(dram)}"
            )
            nc.sync.dma_start(
                t[:],
                dram.ap()[:, g * GS : (g + 1) * GS].rearrange(
                    "(kc p) d -> p kc d", p=128
                ),
            )
            store[n].append(t)

        # consumption order: o-phase of blk0 runs first, so feed it first:
        # x8 b0, all W8o groups (+b1o/w2o), then the f-phase set (W8f/Wr8f
        # groups, xr8 b0, b1f/w2f), then b2s and the remaining x blocks.
        w2_sb = {}
        b1_sb = {}
        b2_sb = {}

        def load_small(n):
            t = consts.tile([128, DC, C], BF16, name=f"w2{n}", tag=f"w2{n}")
            nc.sync.dma_start(t[:], w2_d[n].ap())
            w2_sb[n] = t
            t = consts.tile([128, DC], F32, name=f"b1{n}", tag=f"b1{n}")
            nc.sync.dma_start(t[:], b1_d[n].ap())
            b1_sb[n] = t
            t = consts.tile([128, C], F32, name=f"b2{n}", tag=f"b2{n}")
            nc.sync.dma_start(t[:], b2_d[n].ap())
            b2_sb[n] = t

        load_x_blk(0)
        load_w_group("o", 0, w1_d["o"], w1_sb)
        load_small("o")
        for g in range(1, NG):
            load_w_group("o", g, w1_d["o"], w1_sb)
        if CFG["o"][0] > 0:
            for g in range(NG):
                load_w_group("o", g, wr1_d["o"], wr1_sb)

        load_w_group("f", 0, w1_d["f"], w1_sb)
        if CFG["f"][0] > 0:
            load_w_group("f", 0, wr1_d["f"], wr1_sb)
        if need_xr:
            load_xr_blk(0)
        load_small("f")
        for g in range(1, NG):
            load_w_group("f", g, w1_d["f"], w1_sb)
            if CFG["f"][0] > 0:
                load_w_group("f", g, wr1_d["f"], wr1_sb)

        for blk in range(1, N_MBLK):
            load_x_blk(blk)
            if need_xr:
                load_xr_blk(blk)

        NMH = M_BLK // 256  # 2 moving halves (DoubleRow rhs free limit)

        def layer1_dr(n, blk, dc, ph):
            """fp8 DoubleRow matmuls for one [128 dh x M_BLK] psum tile."""
            cw, cx = CFG[n]
            g, dl = dc // DC_G, dc % DC_G
            dsl = slice(dl * 128, (dl + 1) * 128)
            w8 = w1_sb[n][g]
            terms = [(w8, x8_sb[blk])]
            if cw > 0:
                terms.append((wr1_sb[n][g], x8_sb[blk]))
            if cx > 0:
                terms.append((w8, xr8_sb[blk]))
            for mh in range(NMH):
                msl = slice(mh * 256, (mh + 1) * 256)
                first, last = None, None
                seq = []
                for ti, (wt, xt) in enumerate(terms):
                    climit = KC if ti == 0 else (cw if ti == 1 and cw > 0 else cx)
                    # corrections apply to the FIRST climit chunks
                    for j in range(0, climit, 2):
                        seq.append((wt, xt, j))
                for i, (wt, xt, j) in enumerate(seq):
                    nc.tensor.matmul(
                        ph[:, msl],
                        lhsT=wt[:, j : j + 2, dsl],
                        rhs=xt[:, j : j + 2, msl],
                        start=(i == 0),
                        stop=(i == len(seq) - 1),
                        perf_mode=DR,
                    )

        # o-phase (PE ~440ns/dc) is ACT-bound at ~612ns/dc; route some of its
        # relu tiles to the mostly-idle DVE. The first 3 dc also hide the
        # ACT relu-table reload behind DVE work at each phase start.
        DVE_RELU_DCS = {0, 1, 2, 5, 8, 11, 14, 17, 20, 23}

        # ---- main loop ---------------------------------------------------
        for blk in range(N_MBLK * reps):
            blk = blk % N_MBLK
            m0 = blk * M_BLK
            last_blk = blk == N_MBLK - 1
            probs = {}
            # o always first: comp_max_tau (the long serial chain) feeds off
            # net o, so it overlaps the f-phase even on the last block
            for n in ("o", "f"):
                po = [
                    psum_o.tile([128, C], F32, name=f"po{mm}", tag=f"po{mm}")
                    for mm in range(MM)
                ]
                n_dr = 8 + sum(CFG[n])  # DR products per dc for this net
                dve_dcs = DVE_RELU_DCS

                def emit_l2(dc, hT):
                    for mm in range(MM):
                        nc.tensor.matmul(
                            po[mm][:],
                            lhsT=hT[:, mm * 128 : (mm + 1) * 128],
                            rhs=w2_sb[n][:, dc, :],
                            start=(dc == 0),
                            stop=(dc == DC - 1),
                        )

                pend_l2 = None  # layer-2 is deferred one dc so hT is ready
                for dc in range(DC):
                    ph = psum_h.tile([128, M_BLK], F32, name="ph", tag="ph")
                    layer1_dr(n, blk, dc, ph)
                    hT = hpool.tile([128, M_BLK], BF16, name="hT", tag="hT")
                    if n_dr <= 8 and dc in dve_dcs:
                        t1 = rpool.tile([128, M_BLK], F32, name="t1", tag="t1")
                        nc.vector.scalar_tensor_tensor(
                            t1[:],
                            ph[:],
                            1.0 / W_SCALE,
                            b1_sb[n][:, dc : dc + 1].to_broadcast([128, M_BLK]),
                            Alu.mult,
                            Alu.add,
                        )
                        nc.vector.tensor_scalar(
                            hT[:], t1[:], 0.0, None, Alu.max
                        )
                    else:
                        nc.scalar.activation(
                            hT[:],
                            ph[:],
                            Act.Relu,
                            bias=b1_sb[n][:, dc : dc + 1],
                            scale=1.0 / W_SCALE,
                        )
                    if pend_l2 is not None:
                        emit_l2(*pend_l2)
                    pend_l2 = (dc, hT)
                emit_l2(*pend_l2)
                # softmax over C, batched [128, MM, C]
                z = epool.tile([128, MM, C], F32, name=f"z{n}", tag=f"z{n}")
                negmax = epool.tile([128, MM], F32, name=f"ngm{n}", tag=f"ngm{n}")
                exps = epool.tile([128, MM, C], F32, name=f"ex{n}", tag=f"ex{n}")
                sums = epool.tile([128, MM], F32, name=f"sm{n}", tag=f"sm{n}")
                rinv = epool.tile([128, MM], F32, name=f"ri{n}", tag=f"ri{n}")
                wrk = epool.tile([128, MM], F32, name=f"wk{n}", tag=f"wk{n}")
                for mm in range(MM):
                    nc.vector.tensor_tensor(
                        z[:, mm, :], po[mm][:], b2_sb[n][:], Alu.add
                    )
                for mm in range(MM):
                    nc.vector.tensor_reduce(
                        negmax[:, mm : mm + 1],
                        z[:, mm, :],
                        axis=mybir.AxisListType.X,
                        op=Alu.max,
                        negate=True,
                    )
                for mm in range(MM):
                    nc.scalar.activation(
                        exps[:, mm, :],
                        z[:, mm, :],
                        Act.Exp,
                        bias=negmax[:, mm : mm + 1],
                        accum_out=sums[:, mm : mm + 1],
                    )
                nc.vector.reciprocal(rinv[:], sums[:])
                # one Newton step: r <- r * (2 - s*r)
                nc.vector.tensor_tensor(wrk[:], sums[:], rinv[:], Alu.mult)
                nc.vector.tensor_scalar(wrk[:], wrk[:], -1.0, 2.0, Alu.mult, Alu.add)
                nc.vector.tensor_tensor(rinv[:], rinv[:], wrk[:], Alu.mult)
                pr = epool.tile([128, MM, C], F32, name=f"pr{n}", tag=f"pr{n}")
                nc.vector.tensor_tensor(
                    pr[:],
                    exps[:],
                    rinv[:, :, None].to_broadcast([128, MM, C]),
                    Alu.mult,
                )
                probs[n] = pr

            # ---- comp_max_tau on probs["o"] ------------------------------
            res = epool.tile([128, MM, C + 1], F32, name="res", tag="res")
            s4 = epool.tile([128, MM], F32, name="s4", tag="s4")
            u4 = epool.tile([128, MM], F32, name="u4", tag="u4")
            b4 = epool.tile([128, MM], F32, name="b4", tag="b4")
            a4 = epool.tile([128, MM], F32, name="a4", tag="a4")
            nc.vector.tensor_scalar(
                res[:, :, 0:C], probs["o"][:], 0.0, None, Alu.add
            )
            nc.vector.memset(res[:, :, C : C + 1], TAU)
            for i in range(T1):
                m_i = 2.0 + TAU * TAU if i == 0 else 2.0
                k_i = 2.0 / m_i
                nc.vector.tensor_tensor(res[:], res[:], res[:], Alu.mult)
                nc.vector.tensor_reduce(
                    s4[:], res[:], axis=mybir.AxisListType.X, op=Alu.add
                )
                nc.vector.tensor_scalar(u4[:], s4[:], k_i, None, Alu.mult)
                nc.vector.tensor_scalar(b4[:], u4[:], -1.0, 1.0, Alu.mult, Alu.add)
                nc.vector.tensor_scalar(
                    a4[:], u4[:], -k_i, 2.0 * k_i, Alu.mult, Alu.add
                )
                for _ in range(T2):
                    nc.vector.tensor_tensor(b4[:], b4[:], b4[:], Alu.mult)
                    nc.vector.scalar_tensor_tensor(
                        a4[:], b4[:], 1.0, a4[:], Alu.add, Alu.mult
                    )
                nc.vector.tensor_tensor(
                    res[:],
                    res[:],
                    a4[:, :, None].to_broadcast([128, MM, C + 1]),
                    Alu.mult,
                )

            # ---- blend: out = x1 + cond * (x2 - x1) ----------------------
            dd = epool.tile([128, MM, C], F32, name="dd", tag="dd")
            outt = opool.tile([128, MM, C], F32, name="outt", tag="outt")
            nc.vector.tensor_tensor(dd[:], probs["f"][:], probs["o"][:], Alu.subtract)
            nc.vector.tensor_tensor(
                dd[:],
                dd[:],
                res[:, :, C : C + 1].to_broadcast([128, MM, C]),
                Alu.mult,
            )
            nc.vector.tensor_tensor(outt[:], dd[:], probs["o"][:], Alu.add)
            nc.sync.dma_start(
                out_d.ap()[m0 : m0 + M_BLK, :].rearrange("(mm p) c -> p mm c", p=128),
                outt[:],
            )

    nc.compile()
    return nc


def _get_module():
    if "nc" not in _BUILD_CACHE:
        _BUILD_CACHE["nc"] = _build_module()
    return _BUILD_CACHE["nc"]


def kernel(x, W1o, b1o, W2o, b2o, W1f, b1f, W2f, b2f):
    x = np.asarray(x, dtype=np.float32)
    bf = ml_dtypes.bfloat16
    e4 = ml_dtypes.float8_e4m3

    x8 = x.astype(e4)
    need_xr = any(cfg[1] > 0 for cfg in CFG.values())
    xr8 = (x - x8.astype(np.float32)).astype(e4) if need_xr else None

    w1q = {}
    wr1q = {}
    for n, W1 in (("o", W1o), ("f", W1f)):
        Ws = np.asarray(W1, np.float32) * W_SCALE
        W8 = Ws.astype(e4)
        w1q[n] = np.ascontiguousarray(W8)
        if CFG[n][0] > 0:
            wr1q[n] = np.ascontiguousarray((Ws - W8.astype(np.float32)).astype(e4))
    def pack_w2(W2):
        a = np.asarray(W2, np.float32).astype(bf).reshape(DC, 128, C)
        return np.ascontiguousarray(a.transpose(1, 0, 2))  # [128, DC, C]

    def pack_b1(b):
        a = np.asarray(b, np.float32).reshape(DC, 128)
        return np.ascontiguousarray(a.T)  # [128, DC]

    def pack_b2(b):
        return np.ascontiguousarray(
            np.broadcast_to(np.asarray(b, np.float32), (128, C))
        )

    w2 = {"o": pack_w2(W2o), "f": pack_w2(W2f)}
    b1 = {"o": pack_b1(b1o), "f": pack_b1(b1f)}
    b2 = {"o": pack_b2(b2o), "f": pack_b2(b2f)}

    nc = _get_module()

    in_maps = []
    for i in range(N_CORES):
        sl = slice(i * M, (i + 1) * M)
        m = {"x8T": np.ascontiguousarray(x8[sl].T)}
        if need_xr:
            m["xr8T"] = np.ascontiguousarray(xr8[sl].T)
        for n in ("o", "f"):
            m[f"w1{n}"] = w1q[n]
            if CFG[n][0] > 0:
                m[f"wr1{n}"] = wr1q[n]
            m[f"w2{n}"] = w2[n]
            m[f"b1{n}"] = b1[n]
            m[f"b2{n}"] = b2[n]
        in_maps.append(m)

    trace = bool(os.environ.get("KERNEL_TRACE"))
    results = run_bass_kernel_spmd(
        nc, in_maps, list(range(N_CORES)), trace=trace
    )
    global LAST_RESULTS
    LAST_RESULTS = results

    out = np.concatenate(
        [np.asarray(results.results[i]["out"], np.float32) for i in range(N_CORES)],
        axis=0,
    )
    return out
